# revision 21
# baseline (speedup 1.0000x reference)
"""Causal self-attention (B=4, T=2048, C=1024, H=16, D=64) on 8 trn2 cores.

Sharding: core c -> (batch b = c//2, head-group g = c%2); a head group is
8 heads = 512 feature columns of each of Q/K/V.  Per core, one fully
software-pipelined program:

  - QKV projection blocks produce Q^T/K^T [64,2048] fp16 per head and
    V [2048,64] fp16 (+ a ones column that makes the AV matmul emit the
    softmax denominator for free).
  - Scores stay transposed (S^T[k,q]) so exp(S^T) feeds the AV matmul as
    the moving operand with no transposes anywhere.  On diagonal chunks
    the two heads' valid column ranges are written contiguously so the
    exp is a single wide ACT call, and only the 128-wide causal triangle
    is masked (the fully-masked region is excluded from the matmuls).
  - The attention stream is ACT(exp)-bound, so the next token-block's
    projection matmuls and earlier query-blocks' out-projections are
    drip-fed as 2-matmul micro-chunks inside the attention kc-loop to
    fill TensorE slack.

Host pre-arranges inputs partition-major (fp16) and sums the two per-batch
partials, folding b_out + b_v @ W_out (exact: softmax rows sum to 1).

All matmuls run fp16 with fp32 PSUM accumulation.
"""

from collections import deque
from contextlib import ExitStack

import numpy as np

import concourse.bass as bass
import concourse.mybir as mybir
import concourse.tile as tile
from concourse import bacc
from concourse import bass_utils

F32 = mybir.dt.float32
F16 = mybir.dt.float16

B, T, C = 4, 2048, 1024
H, D = 16, 64
G = 2            # head groups (cores per batch)
HPG = 8          # heads per group
CPH = HPG * D    # feature columns per group = 512
N = 512          # matmul moving free dim
NCORES = 8
SCALE = 1.0 / np.sqrt(D)

_CACHE = {}


def _build_program():
    if "nc" in _CACHE:
        return _CACHE["nc"]

    nc = bacc.Bacc("TRN2", target_bir_lowering=False, debug=False, num_devices=NCORES)

    # all inputs pre-arranged host-side: partition-major, fp16.
    # x is stored token-block-major so each token block is one DMA and the
    # first projection block only depends on the first x DMA.
    xTr = nc.dram_tensor("xTr", [4, 128, 8, N], F16, kind="ExternalInput").ap()
    wqr = nc.dram_tensor("wqr", [128, 8, CPH], F16, kind="ExternalInput").ap()
    wkr = nc.dram_tensor("wkr", [128, 8, CPH], F16, kind="ExternalInput").ap()
    wvr = nc.dram_tensor("wvr", [128, 8, CPH], F16, kind="ExternalInput").ap()
    bqr = nc.dram_tensor("bqr", [128, 4], F32, kind="ExternalInput").ap()
    bkr = nc.dram_tensor("bkr", [128, 4], F32, kind="ExternalInput").ap()
    wor = nc.dram_tensor("wor", [128, 4, C], F16, kind="ExternalInput").ap()
    masks = nc.dram_tensor("masks", [128, 256], F16, kind="ExternalInput").ap()
    yp = nc.dram_tensor("yp", [T, C], F16, kind="ExternalOutput").ap()

    with tile.TileContext(nc) as tc, ExitStack() as ctx:
        wpool = ctx.enter_context(tc.tile_pool(name="wpool", bufs=1))
        big = ctx.enter_context(tc.tile_pool(name="big", bufs=1))
        epool = ctx.enter_context(tc.tile_pool(name="et", bufs=6))
        mpool = ctx.enter_context(tc.tile_pool(name="mpool", bufs=4))
        blkps = ctx.enter_context(tc.tile_pool(name="blkps", bufs=2, space="PSUM"))
        sps = ctx.enter_context(tc.tile_pool(name="sps", bufs=2, space="PSUM"))
        avps = ctx.enter_context(tc.tile_pool(name="avps", bufs=1, space="PSUM"))

        XT = [big.tile([128, 8, N], F16, name=f"xt{tb}") for tb in range(4)]
        QT = big.tile([128, 4, T], F16)   # Q^T (+bias)
        KT = big.tile([128, 4, T], F16)   # SCALE * (K^T + bias)
        VA = big.tile([128, 16, HPG, D + 1], F16)   # V rows + ones column
        ON = big.tile([128, 4, T], F16)   # normalized O^T (c_in x tokens)

        WQ = wpool.tile([128, 8, CPH], F16)
        WK = wpool.tile([128, 8, CPH], F16)
        WV = wpool.tile([128, 8, CPH], F16)
        BQ = wpool.tile([128, 4], F32)
        BKs = wpool.tile([128, 4], F32)
        MS = wpool.tile([128, 256], F16)
        WO = wpool.tile([128, 4, C], F16)

        WARM = wpool.tile([128, N], F16)  # operand for PE warm-up matmuls
        # issued while the first input DMAs stream in, so the HAM
        # clock-gate is at 8/8 when real matmuls start
        nc.any.memset(WARM[:], 1.0)
        warmps = blkps.tile([128, N], F32, name="blk")
        for _ in range(38):
            nc.tensor.matmul(
                warmps[:], WARM[:, 0:128], WARM[:], start=True, stop=True,
                skip_group_check=True,
            )

        # input DMAs in first-use order: the first projection block (q, tb0)
        # needs only WQ + XT[0]
        nc.sync.dma_start(WQ[:], wqr)
        nc.sync.dma_start(XT[0][:], xTr[0])
        nc.sync.dma_start(WK[:], wkr)
        nc.sync.dma_start(WV[:], wvr)
        nc.sync.dma_start(BQ[:], bqr)
        nc.sync.dma_start(BKs[:], bkr)
        nc.vector.tensor_scalar_mul(BKs[:], BKs[:], SCALE)
        nc.sync.dma_start(MS[:], masks)
        nc.any.memset(VA[:, :, :, D : D + 1], 1.0)
        nc.sync.dma_start(XT[1][:], xTr[1])
        nc.sync.dma_start(XT[2][:], xTr[2])
        nc.sync.dma_start(WO[:], wor)
        nc.sync.dma_start(XT[3][:], xTr[3])

        def qkv_block_gen(tb, which, dc):
            """Generator: one projection block, yielding every 2 matmuls."""
            ps = blkps.tile([128, N], F32, name="blk")
            if which == "v":
                for cc in range(8):
                    nc.tensor.matmul(
                        ps[:],
                        XT[tb][:, cc, dc * 128 : (dc + 1) * 128],
                        WV[:, cc],
                        start=(cc == 0),
                        stop=(cc == 7),
                    )
                    if cc % 2 == 1:
                        yield
                nc.vector.tensor_copy(
                    VA[:, tb * 4 + dc, :, 0:D],
                    ps[:].rearrange("p (h d) -> p h d", h=HPG),
                )
            else:
                WT, dst, scl, bias = (
                    (WQ, QT, 1.0, BQ) if which == "q" else (WK, KT, SCALE, BKs)
                )
                for cc in range(8):
                    nc.tensor.matmul(
                        ps[:],
                        WT[:, cc, dc * 128 : (dc + 1) * 128],
                        XT[tb][:, cc, :],
                        start=(cc == 0),
                        stop=(cc == 7),
                    )
                    if cc % 2 == 1:
                        yield
                nc.vector.scalar_tensor_tensor(
                    out=dst[:, dc, tb * N : (tb + 1) * N],
                    in0=ps[:],
                    scalar=scl,
                    in1=bias[:, dc, None].to_broadcast((128, N)),
                    op0=mybir.AluOpType.mult,
                    op1=mybir.AluOpType.add,
                )

        def y_block_gen(ic, ob):
            """Generator: one out-projection block, yielding every 2 matmuls."""
            ypt = blkps.tile([128, N], F32, name="blk")
            for cc4 in range(4):
                nc.tensor.matmul(
                    ypt[:],
                    ON[:, cc4, ic * 128 : (ic + 1) * 128],
                    WO[:, cc4, ob * N : (ob + 1) * N],
                    start=(cc4 == 0),
                    stop=(cc4 == 3),
                )
                if cc4 % 2 == 1:
                    yield
            ysb = mpool.tile([128, N], F16, name="ysb")
            nc.vector.tensor_copy(ysb[:], ypt[:])
            nc.sync.dma_start(
                yp[ic * 128 : (ic + 1) * 128, ob * N : (ob + 1) * N], ysb[:]
            )

        drip = deque()

        def drip_advance(n):
            for _ in range(n):
                while drip:
                    try:
                        next(drip[0])
                        break
                    except StopIteration:
                        drip.popleft()
                else:
                    return

        def drip_drain():
            while drip:
                drip_advance(1)

        def drain_until(g):
            # run the drip until generator g (already in the deque) has
            # completed, so everything it emits lands in program order
            # before the instruction that depends on it
            while g in drip:
                drip_advance(1)

        def ph1_gens(tb):
            # Q chunks first (needed at the start of query-block tb), then
            # K and V (needed from kc=4*tb onwards)
            return (
                [qkv_block_gen(tb, "q", dc) for dc in range(4)]
                + [qkv_block_gen(tb, "k", dc) for dc in range(4)]
                + [qkv_block_gen(tb, "v", dc) for dc in range(4)]
            )

        def y_gens(qi):
            return [
                y_block_gen(4 * qi + i4, ob) for i4 in range(4) for ob in range(2)
            ]

        # token-block 0 preamble: only the blocks attention (qi=0, pr=0)
        # needs; the remaining head-pairs' q/k blocks drip inside qi=0.
        pre0 = [("q", 0), ("k", 0), ("v", 0), ("v", 1), ("v", 2), ("v", 3)]
        drip.extend(qkv_block_gen(0, w, dc) for w, dc in pre0)
        drip_drain()
        qk0 = {(w, dc): qkv_block_gen(0, w, dc)
               for w, dc in [("q", 1), ("k", 1), ("q", 2), ("k", 2),
                             ("q", 3), ("k", 3)]}
        drip.extend(qk0.values())
        q3 = {}
        kv3 = {}

        # drip rate per attention iteration, as a fraction (num/den).
        # qi=0/1 are PE-bound (mandatory next-token-block projections), so
        # the out-projection backlog is deferred to qi=2/3 where the
        # ACT-paced attention stream leaves TensorE slack.
        # Supply (generator next()-calls: a qkv block is 5, a y block is 3):
        #   qi=0 (16 it): rest of tb0 (30) + tb1 (60)
        #   qi=1 (32 it): tb2 (60)
        #   qi=2 (48 it): tb3 q-blocks (20) + y(qi0) (24) + y(qi1) (24)
        #   qi=3 (64 it): tb3 k/v (40, first used at kc=12) + y(qi2) (24)
        DRIP_BUDGET = {0: (6, 1), 1: (2, 1), 2: (3, 2), 3: (1, 1)}
        for qi in range(4):
            nkc = 4 * qi + 4
            if qi in (0, 1):
                drip_drain()  # safety: previous token block must be complete
                drip.extend(ph1_gens(qi + 1))
            elif qi == 2:
                drip_drain()
                # only tb3's Q is needed at the start of qi=3; its K/V
                # blocks drip inside qi=3 (first used at kc=12)
                q3 = {dc: qkv_block_gen(3, "q", dc) for dc in range(4)}
                drip.extend(q3.values())
                drip.extend(y_gens(0))
                drip.extend(y_gens(1))
            else:
                kv3 = {(w, dc): qkv_block_gen(3, w, dc)
                       for w, dc in [("k", 0), ("v", 0), ("v", 1), ("v", 2),
                                     ("v", 3), ("k", 1), ("k", 2), ("k", 3)]}
                drip.extend(kv3.values())
                drip.extend(y_gens(2))
            bnum, bden = DRIP_BUDGET[qi]
            bacc_ctr = 0
            for pr in range(4):
                # force-complete the projection blocks this head-pair's
                # attention reads, so their instructions precede ours
                if qi == 0 and pr > 0:
                    drain_until(qk0[("q", pr)])
                    drain_until(qk0[("k", pr)])
                if qi == 3:
                    drain_until(q3[pr])
                avs = []
                for hi in range(2):
                    av = avps.tile([D + 1, N], F32, name=f"av{hi}")
                    avs.append(av)

                def emit_s(kc, pr=pr, qi=qi):
                    # both heads' score tiles in one 2-bank psum tile so the
                    # exp runs wide; the two matmuls run concurrently (row
                    # groups 0-1 / 2-3).  Diagonal chunks only compute the
                    # causally-reachable range w = N - vq; head 0 is packed
                    # at the END of psum bank 0 ([N-w, N)) and head 1 at the
                    # START of bank 1 ([N, N+w)) so the valid region is one
                    # contiguous [N-w, N+w) stripe while each matmul output
                    # stays inside a single psum bank.
                    vq = max(0, (kc - 4 * qi) * 128)
                    w = N - vq
                    sp = sps.tile([128, 2 * N], F32, name="sp")
                    for hi in range(2):
                        off = 64 * hi
                        nc.tensor.matmul(
                            sp[:, N - w + hi * w : N + hi * w],
                            KT[off : off + 64, pr, kc * 128 : (kc + 1) * 128],
                            QT[off : off + 64, pr, qi * N + vq : (qi + 1) * N],
                            start=True,
                            stop=True,
                        )
                    return sp

                sp_cur = emit_s(0)
                for kc in range(nkc):
                    if qi == 3 and 12 <= kc + 1 < nkc:
                        # tb3 K/V blocks drip inside qi=3; force-complete
                        # the ones the next chunk's S and AV read
                        if kc + 1 == 12:
                            drain_until(kv3[("k", pr)])
                        drain_until(kv3[("v", kc + 1 - 12)])
                    # S(kc+1) first: it has no dependency on this chunk's
                    # exp/mask, so it streams during the et latency instead
                    # of head-of-line blocking behind the AV matmuls
                    sp_next = emit_s(kc + 1) if kc + 1 < nkc else None
                    et = epool.tile([128, 2 * N], F16, name="et")
                    vq = max(0, (kc - 4 * qi) * 128)
                    w = N - vq
                    nc.scalar.activation(
                        et[:, N - w : N + w],
                        sp_cur[:, N - w : N + w],
                        mybir.ActivationFunctionType.Exp,
                    )
                    if kc >= 4 * qi:
                        # only the leading 128 columns of each head's range
                        # are partially masked (local query j vs key k:
                        # masked iff j < k); the rest is fully visible.
                        if w == 128:
                            nc.vector.tensor_tensor(
                                et[:, N - 128 : N + 128],
                                et[:, N - 128 : N + 128],
                                MS[:],
                                mybir.AluOpType.mult,
                            )
                        else:
                            for hi in range(2):
                                lo = N - w + hi * w
                                nc.vector.tensor_tensor(
                                    et[:, lo : lo + 128],
                                    et[:, lo : lo + 128],
                                    MS[:, 0:128],
                                    mybir.AluOpType.mult,
                                )
                    for hi in range(2):
                        nc.tensor.matmul(
                            avs[hi][:, vq:],
                            VA[:, kc, 2 * pr + hi, :],
                            et[:, N - w + hi * w : N + hi * w],
                            start=(kc == 0),
                            stop=(kc == nkc - 1),
                        )
                    bacc_ctr += bnum
                    drip_advance(bacc_ctr // bden)
                    bacc_ctr %= bden
                    sp_cur = sp_next

                last_pr = qi == 3 and pr == 3
                for hi in range(2):
                    off = 64 * hi
                    if last_pr:
                        # nothing reuses the accumulator banks after this,
                        # so skip the bank-releasing copy and read PSUM
                        # directly — shortens the final-normalize tail
                        oc = avs[hi]
                    else:
                        # one copy releases the accumulator bank; the rest
                        # of the chain runs off SBUF, off the critical path
                        oc = mpool.tile([D + 1, N], F32, name="oc")
                        nc.vector.tensor_copy(oc[:], avs[hi][:])
                    dn = mpool.tile([1, N], F32, name="dn")
                    nc.vector.tensor_copy(dn[:], oc[D : D + 1, :])
                    rd = mpool.tile([1, N], F32, name="rd")
                    nc.vector.reciprocal_approx_fast(rd[:], dn[:])
                    rb = mpool.tile([64, N], F32, name="rb")
                    nc.gpsimd.partition_broadcast(rb[:], rd[:])
                    seg = ON[off : off + 64, pr, qi * N : (qi + 1) * N]
                    nc.vector.tensor_tensor(
                        seg, oc[0:64, :], rb[:], mybir.AluOpType.mult
                    )
        drip.extend(y_gens(3))
        drip_drain()

    nc.compile()
    _CACHE["nc"] = nc
    return nc


def _make_masks():
    kp = np.arange(128)[:, None]
    qf = np.arange(128)[None, :]
    tri = (qf >= kp)
    m = np.concatenate([tri, tri], axis=1)  # [128, 256]
    return np.ascontiguousarray(m.astype(np.float16))


def _pm(a, chunks):
    """[chunks*128, F] -> partition-major [128, chunks, F] fp16, contiguous."""
    f = a.shape[-1]
    return np.ascontiguousarray(
        a.reshape(chunks, 128, f).transpose(1, 0, 2).astype(np.float16)
    )


def _make_in_maps(x, W_qkv, b_qkv, W_out):
    x = np.asarray(x, dtype=np.float32)
    W_qkv = np.asarray(W_qkv, dtype=np.float32)
    b_qkv = np.asarray(b_qkv, dtype=np.float32)
    W_out = np.asarray(W_out, dtype=np.float32)
    masks = _make_masks()
    # x[b].T is [C, T]; layout [tb, p, cc, t] with C = cc*128 + p,
    # T = tb*512 + t
    xTr = [
        np.ascontiguousarray(
            x[b].T.reshape(8, 128, 4, N).transpose(2, 1, 0, 3).astype(np.float16)
        )
        for b in range(B)
    ]
    in_maps = []
    for c in range(NCORES):
        b, g = c // G, c % G
        lo = CPH * g
        bqr = np.ascontiguousarray(
            b_qkv[lo : lo + CPH].reshape(4, 128).T.astype(np.float32)
        )
        bkr = np.ascontiguousarray(
            b_qkv[C + lo : C + lo + CPH].reshape(4, 128).T.astype(np.float32)
        )
        in_maps.append(
            {
                "xTr": xTr[b],
                "wqr": _pm(W_qkv[:, lo : lo + CPH], 8),
                "wkr": _pm(W_qkv[:, C + lo : C + lo + CPH], 8),
                "wvr": _pm(W_qkv[:, 2 * C + lo : 2 * C + lo + CPH], 8),
                "bqr": bqr,
                "bkr": bkr,
                "wor": _pm(W_out[lo : lo + CPH, :], 4),
                "masks": masks,
            }
        )
    return in_maps


def _gather(results, b_out, bias_extra):
    bias = np.asarray(b_out, dtype=np.float32) + bias_extra
    out = np.empty((B, T, C), np.float32)
    for b in range(B):
        out[b] = (
            results[G * b]["yp"].astype(np.float32)
            + results[G * b + 1]["yp"].astype(np.float32)
            + bias[None, :]
        )
    return out


def kernel(x, W_qkv, b_qkv, W_out, b_out, **_):
    nc = _build_program()
    in_maps = _make_in_maps(x, W_qkv, b_qkv, W_out)
    res = bass_utils.run_bass_kernel_spmd(nc, in_maps, core_ids=list(range(NCORES)))
    bias_extra = np.asarray(b_qkv, np.float32)[2 * C :] @ np.asarray(W_out, np.float32)
    return _gather(res.results, b_out, bias_extra)


def kernel_traced(x, W_qkv, b_qkv, W_out, b_out, tmpdir=None, trace=True, **_):
    """Like kernel() but returns (out, exec_time_ns); used by test.py."""
    nc = _build_program()
    in_maps = _make_in_maps(x, W_qkv, b_qkv, W_out)
    res = bass_utils.run_bass_kernel_spmd(
        nc, in_maps, core_ids=list(range(NCORES)), trace=trace, tmpdir=tmpdir
    )
    bias_extra = np.asarray(b_qkv, np.float32)[2 * C :] @ np.asarray(W_out, np.float32)
    return _gather(res.results, b_out, bias_extra), res.exec_time_ns


# revision 24
# speedup vs baseline: 1.1849x; 1.1849x over previous
"""Causal self-attention (B=4, T=2048, C=1024, H=16, D=64) on 8 trn2 cores.

Sharding: core c -> (batch b = c//2, head-group g = c%2); a head group is
8 heads = 512 feature columns of each of Q/K/V.  Per core, one fully
software-pipelined program:

  - QKV projection blocks produce Q^T/K^T [64,2048] fp16 per head and
    V [2048,64] fp16 (+ a ones column that makes the AV matmul emit the
    softmax denominator for free).
  - Scores stay transposed (S^T[k,q]) so exp(S^T) feeds the AV matmul as
    the moving operand with no transposes anywhere.  On diagonal chunks
    the two heads' valid column ranges are written contiguously so the
    exp is a single wide ACT call, and only the 128-wide causal triangle
    is masked (the fully-masked region is excluded from the matmuls).
  - The attention stream is ACT(exp)-bound, so the next token-block's
    projection matmuls and earlier query-blocks' out-projections are
    drip-fed as 2-matmul micro-chunks inside the attention kc-loop to
    fill TensorE slack.

Host pre-arranges inputs partition-major (fp16) and sums the two per-batch
partials, folding b_out + b_v @ W_out (exact: softmax rows sum to 1).

All matmuls run fp16 with fp32 PSUM accumulation.
"""

from collections import deque
from contextlib import ExitStack

import numpy as np

import concourse.bass as bass
import concourse.mybir as mybir
import concourse.tile as tile
from concourse import bacc
from concourse import bass_utils

F32 = mybir.dt.float32
F16 = mybir.dt.float16

B, T, C = 4, 2048, 1024
H, D = 16, 64
G = 2            # head groups (cores per batch)
HPG = 8          # heads per group
CPH = HPG * D    # feature columns per group = 512
N = 512          # matmul moving free dim
NCORES = 8
SCALE = 1.0 / np.sqrt(D)

_CACHE = {}


def _build_program():
    if "nc" in _CACHE:
        return _CACHE["nc"]

    nc = bacc.Bacc("TRN2", target_bir_lowering=False, debug=False, num_devices=NCORES)

    # all inputs pre-arranged host-side: partition-major, fp16.
    # x is stored token-block-major so each token block is one DMA and the
    # first projection block only depends on the first x DMA.
    xTr = nc.dram_tensor("xTr", [4, 128, 8, N], F16, kind="ExternalInput").ap()
    wqr = nc.dram_tensor("wqr", [128, 8, CPH], F16, kind="ExternalInput").ap()
    wkr = nc.dram_tensor("wkr", [128, 8, CPH], F16, kind="ExternalInput").ap()
    wvr = nc.dram_tensor("wvr", [128, 8, CPH], F16, kind="ExternalInput").ap()
    bqr = nc.dram_tensor("bqr", [128, 4], F32, kind="ExternalInput").ap()
    bkr = nc.dram_tensor("bkr", [128, 4], F32, kind="ExternalInput").ap()
    wor = nc.dram_tensor("wor", [128, 4, C], F16, kind="ExternalInput").ap()
    masks = nc.dram_tensor("masks", [128, 256], F16, kind="ExternalInput").ap()
    yp = nc.dram_tensor("yp", [T, C], F16, kind="ExternalOutput").ap()

    with tile.TileContext(nc) as tc, ExitStack() as ctx:
        wpool = ctx.enter_context(tc.tile_pool(name="wpool", bufs=1))
        big = ctx.enter_context(tc.tile_pool(name="big", bufs=1))
        epool = ctx.enter_context(tc.tile_pool(name="et", bufs=6))
        mpool = ctx.enter_context(tc.tile_pool(name="mpool", bufs=4))
        blkps = ctx.enter_context(tc.tile_pool(name="blkps", bufs=2, space="PSUM"))
        sps = ctx.enter_context(tc.tile_pool(name="sps", bufs=2, space="PSUM"))
        avps = ctx.enter_context(tc.tile_pool(name="avps", bufs=1, space="PSUM"))

        XT = [big.tile([128, 8, N], F16, name=f"xt{tb}") for tb in range(4)]
        QT = big.tile([128, 4, T], F16)   # Q^T (+bias)
        KT = big.tile([128, 4, T], F16)   # SCALE * (K^T + bias)
        VA = big.tile([128, 16, HPG, D + 1], F16)   # V rows + ones column
        ON = big.tile([128, 4, T], F16)   # normalized O^T (c_in x tokens)

        WQ = wpool.tile([128, 8, CPH], F16)
        WK = wpool.tile([128, 8, CPH], F16)
        WV = wpool.tile([128, 8, CPH], F16)
        BQ = wpool.tile([128, 4], F32)
        BKs = wpool.tile([128, 4], F32)
        MS = wpool.tile([128, 256], F16)
        WO = wpool.tile([128, 4, C], F16)

        WARM = wpool.tile([128, N], F16)  # operand for PE warm-up matmuls
        # issued while the first input DMAs stream in, so the HAM
        # clock-gate is at 8/8 when real matmuls start
        nc.any.memset(WARM[:], 1.0)
        warmps = blkps.tile([128, N], F32, name="blk")
        for _ in range(38):
            nc.tensor.matmul(
                warmps[:], WARM[:, 0:128], WARM[:], start=True, stop=True,
                skip_group_check=True,
            )

        # input DMAs in first-use order: the first projection block (q, tb0)
        # needs only WQ + XT[0]
        nc.sync.dma_start(WQ[:], wqr)
        nc.sync.dma_start(XT[0][:], xTr[0])
        nc.sync.dma_start(WK[:], wkr)
        nc.sync.dma_start(WV[:], wvr)
        nc.sync.dma_start(BQ[:], bqr)
        nc.sync.dma_start(BKs[:], bkr)
        nc.vector.tensor_scalar_mul(BKs[:], BKs[:], SCALE)
        # masks split in two so XT[1] is the 9th DMA: it then waits for a
        # free hardware queue slot instead of stealing input bandwidth
        # from WK/WV during the startup-critical window
        nc.sync.dma_start(MS[:, 0:128], masks[:, 0:128])
        nc.sync.dma_start(MS[:, 128:256], masks[:, 128:256])
        nc.any.memset(VA[:, :, :, D : D + 1], 1.0)
        nc.sync.dma_start(XT[1][:], xTr[1])
        nc.sync.dma_start(XT[2][:], xTr[2])
        nc.sync.dma_start(WO[:], wor)
        nc.sync.dma_start(XT[3][:], xTr[3])

        def qkv_block_gen(tb, which, dc):
            """Generator: one projection block, yielding every 2 matmuls."""
            ps = blkps.tile([128, N], F32, name="blk")
            if which == "v":
                for cc in range(8):
                    nc.tensor.matmul(
                        ps[:],
                        XT[tb][:, cc, dc * 128 : (dc + 1) * 128],
                        WV[:, cc],
                        start=(cc == 0),
                        stop=(cc == 7),
                    )
                    if cc % 2 == 1:
                        yield
                nc.vector.tensor_copy(
                    VA[:, tb * 4 + dc, :, 0:D],
                    ps[:].rearrange("p (h d) -> p h d", h=HPG),
                )
            else:
                WT, dst, scl, bias = (
                    (WQ, QT, 1.0, BQ) if which == "q" else (WK, KT, SCALE, BKs)
                )
                for cc in range(8):
                    nc.tensor.matmul(
                        ps[:],
                        WT[:, cc, dc * 128 : (dc + 1) * 128],
                        XT[tb][:, cc, :],
                        start=(cc == 0),
                        stop=(cc == 7),
                    )
                    if cc % 2 == 1:
                        yield
                nc.vector.scalar_tensor_tensor(
                    out=dst[:, dc, tb * N : (tb + 1) * N],
                    in0=ps[:],
                    scalar=scl,
                    in1=bias[:, dc, None].to_broadcast((128, N)),
                    op0=mybir.AluOpType.mult,
                    op1=mybir.AluOpType.add,
                )

        def y_block_gen(ic, ob):
            """Generator: one out-projection block, yielding every 2 matmuls."""
            ypt = blkps.tile([128, N], F32, name="blk")
            for cc4 in range(4):
                nc.tensor.matmul(
                    ypt[:],
                    ON[:, cc4, ic * 128 : (ic + 1) * 128],
                    WO[:, cc4, ob * N : (ob + 1) * N],
                    start=(cc4 == 0),
                    stop=(cc4 == 3),
                )
                if cc4 % 2 == 1:
                    yield
            ysb = mpool.tile([128, N], F16, name="ysb")
            nc.vector.tensor_copy(ysb[:], ypt[:])
            nc.sync.dma_start(
                yp[ic * 128 : (ic + 1) * 128, ob * N : (ob + 1) * N], ysb[:]
            )

        drip = deque()

        def drip_advance(n):
            for _ in range(n):
                while drip:
                    try:
                        next(drip[0])
                        break
                    except StopIteration:
                        drip.popleft()
                else:
                    return

        def drip_drain():
            while drip:
                drip_advance(1)

        def drain_until(g):
            # run the drip until generator g (already in the deque) has
            # completed, so everything it emits lands in program order
            # before the instruction that depends on it
            while g in drip:
                drip_advance(1)

        def ph1_gens(tb):
            # Q chunks first (needed at the start of query-block tb), then
            # K and V (needed from kc=4*tb onwards)
            return (
                [qkv_block_gen(tb, "q", dc) for dc in range(4)]
                + [qkv_block_gen(tb, "k", dc) for dc in range(4)]
                + [qkv_block_gen(tb, "v", dc) for dc in range(4)]
            )

        def y_gens(qi):
            return [
                y_block_gen(4 * qi + i4, ob) for i4 in range(4) for ob in range(2)
            ]

        # token-block 0 preamble: only the blocks attention (qi=0, pr=0)
        # needs; the remaining head-pairs' q/k blocks drip inside qi=0.
        pre0 = [("q", 0), ("k", 0), ("v", 0), ("v", 1), ("v", 2), ("v", 3)]
        drip.extend(qkv_block_gen(0, w, dc) for w, dc in pre0)
        drip_drain()
        qk0 = {(w, dc): qkv_block_gen(0, w, dc)
               for w, dc in [("q", 1), ("k", 1), ("q", 2), ("k", 2),
                             ("q", 3), ("k", 3)]}
        drip.extend(qk0.values())
        q3 = {}
        kv3 = {}

        # drip rate per attention iteration, as a fraction (num/den).
        # qi=0/1 are PE-bound (mandatory next-token-block projections), so
        # the out-projection backlog is deferred to qi=2/3 where the
        # ACT-paced attention stream leaves TensorE slack.
        # Supply (generator next()-calls: a qkv block is 5, a y block is 3):
        #   qi=0 (16 it): rest of tb0 (30) + tb1 (60)
        #   qi=1 (32 it): tb2 (60)
        #   qi=2 (48 it): tb3 q-blocks (20) + y(qi0) (24) + y(qi1) (24)
        #   qi=3 (64 it): tb3 k/v (40, first used at kc=12) + y(qi2) (24)
        DRIP_BUDGET = {0: (6, 1), 1: (2, 1), 2: (3, 2), 3: (1, 1)}
        for qi in range(4):
            nkc = 4 * qi + 4
            if qi in (0, 1):
                drip_drain()  # safety: previous token block must be complete
                drip.extend(ph1_gens(qi + 1))
            elif qi == 2:
                drip_drain()
                # only tb3's Q is needed at the start of qi=3; its K/V
                # blocks drip inside qi=3 (first used at kc=12)
                q3 = {dc: qkv_block_gen(3, "q", dc) for dc in range(4)}
                drip.extend(q3.values())
                drip.extend(y_gens(0))
                drip.extend(y_gens(1))
            else:
                kv3 = {(w, dc): qkv_block_gen(3, w, dc)
                       for w, dc in [("k", 0), ("v", 0), ("v", 1), ("v", 2),
                                     ("v", 3), ("k", 1), ("k", 2), ("k", 3)]}
                drip.extend(kv3.values())
                drip.extend(y_gens(2))
            bnum, bden = DRIP_BUDGET[qi]
            bacc_ctr = 0
            for pr in range(4):
                # force-complete the projection blocks this head-pair's
                # attention reads, so their instructions precede ours
                if qi == 0 and pr > 0:
                    drain_until(qk0[("q", pr)])
                    drain_until(qk0[("k", pr)])
                if qi == 3:
                    drain_until(q3[pr])
                avs = []
                for hi in range(2):
                    av = avps.tile([D + 1, N], F32, name=f"av{hi}")
                    avs.append(av)

                def emit_s(kc, pr=pr, qi=qi):
                    # both heads' score tiles in one 2-bank psum tile so the
                    # exp runs wide; the two matmuls run concurrently (row
                    # groups 0-1 / 2-3).  Diagonal chunks only compute the
                    # causally-reachable range w = N - vq; head 0 is packed
                    # at the END of psum bank 0 ([N-w, N)) and head 1 at the
                    # START of bank 1 ([N, N+w)) so the valid region is one
                    # contiguous [N-w, N+w) stripe while each matmul output
                    # stays inside a single psum bank.
                    vq = max(0, (kc - 4 * qi) * 128)
                    w = N - vq
                    sp = sps.tile([128, 2 * N], F32, name="sp")
                    for hi in range(2):
                        off = 64 * hi
                        nc.tensor.matmul(
                            sp[:, N - w + hi * w : N + hi * w],
                            KT[off : off + 64, pr, kc * 128 : (kc + 1) * 128],
                            QT[off : off + 64, pr, qi * N + vq : (qi + 1) * N],
                            start=True,
                            stop=True,
                        )
                    return sp

                sp_cur = emit_s(0)
                for kc in range(nkc):
                    et = epool.tile([128, 2 * N], F16, name="et")
                    vq = max(0, (kc - 4 * qi) * 128)
                    w = N - vq
                    nc.scalar.activation(
                        et[:, N - w : N + w],
                        sp_cur[:, N - w : N + w],
                        mybir.ActivationFunctionType.Exp,
                    )
                    if kc >= 4 * qi:
                        # only the leading 128 columns of each head's range
                        # are partially masked (local query j vs key k:
                        # masked iff j < k); the rest is fully visible.
                        if w == 128:
                            nc.vector.tensor_tensor(
                                et[:, N - 128 : N + 128],
                                et[:, N - 128 : N + 128],
                                MS[:],
                                mybir.AluOpType.mult,
                            )
                        else:
                            for hi in range(2):
                                lo = N - w + hi * w
                                nc.vector.tensor_tensor(
                                    et[:, lo : lo + 128],
                                    et[:, lo : lo + 128],
                                    MS[:, 0:128],
                                    mybir.AluOpType.mult,
                                )
                    for hi in range(2):
                        nc.tensor.matmul(
                            avs[hi][:, vq:],
                            VA[:, kc, 2 * pr + hi, :],
                            et[:, N - w + hi * w : N + hi * w],
                            start=(kc == 0),
                            stop=(kc == nkc - 1),
                        )
                    if qi == 3 and 12 <= kc + 1 < nkc:
                        # tb3 K/V blocks drip inside qi=3; force-complete
                        # the ones the next chunk's S and AV read
                        if kc + 1 == 12:
                            drain_until(kv3[("k", pr)])
                        drain_until(kv3[("v", kc + 1 - 12)])
                    sp_cur = emit_s(kc + 1) if kc + 1 < nkc else None
                    bacc_ctr += bnum
                    drip_advance(bacc_ctr // bden)
                    bacc_ctr %= bden

                last_pr = qi == 3 and pr == 3
                for hi in range(2):
                    off = 64 * hi
                    if last_pr:
                        # nothing reuses the accumulator banks after this,
                        # so skip the bank-releasing copy and read PSUM
                        # directly — shortens the final-normalize tail
                        oc = avs[hi]
                    else:
                        # one copy releases the accumulator bank; the rest
                        # of the chain runs off SBUF, off the critical path
                        oc = mpool.tile([D + 1, N], F32, name="oc")
                        nc.vector.tensor_copy(oc[:], avs[hi][:])
                    dn = mpool.tile([1, N], F32, name="dn")
                    nc.vector.tensor_copy(dn[:], oc[D : D + 1, :])
                    rd = mpool.tile([1, N], F32, name="rd")
                    nc.vector.reciprocal_approx_fast(rd[:], dn[:])
                    rb = mpool.tile([64, N], F32, name="rb")
                    nc.gpsimd.partition_broadcast(rb[:], rd[:])
                    seg = ON[off : off + 64, pr, qi * N : (qi + 1) * N]
                    nc.vector.tensor_tensor(
                        seg, oc[0:64, :], rb[:], mybir.AluOpType.mult
                    )
        drip.extend(y_gens(3))
        drip_drain()

    nc.compile()
    _CACHE["nc"] = nc
    return nc


def _make_masks():
    kp = np.arange(128)[:, None]
    qf = np.arange(128)[None, :]
    tri = (qf >= kp)
    m = np.concatenate([tri, tri], axis=1)  # [128, 256]
    return np.ascontiguousarray(m.astype(np.float16))


def _pm(a, chunks):
    """[chunks*128, F] -> partition-major [128, chunks, F] fp16, contiguous."""
    f = a.shape[-1]
    return np.ascontiguousarray(
        a.reshape(chunks, 128, f).transpose(1, 0, 2).astype(np.float16)
    )


def _make_in_maps(x, W_qkv, b_qkv, W_out):
    x = np.asarray(x, dtype=np.float32)
    W_qkv = np.asarray(W_qkv, dtype=np.float32)
    b_qkv = np.asarray(b_qkv, dtype=np.float32)
    W_out = np.asarray(W_out, dtype=np.float32)
    masks = _make_masks()
    # x[b].T is [C, T]; layout [tb, p, cc, t] with C = cc*128 + p,
    # T = tb*512 + t
    xTr = [
        np.ascontiguousarray(
            x[b].T.reshape(8, 128, 4, N).transpose(2, 1, 0, 3).astype(np.float16)
        )
        for b in range(B)
    ]
    in_maps = []
    for c in range(NCORES):
        b, g = c // G, c % G
        lo = CPH * g
        bqr = np.ascontiguousarray(
            b_qkv[lo : lo + CPH].reshape(4, 128).T.astype(np.float32)
        )
        bkr = np.ascontiguousarray(
            b_qkv[C + lo : C + lo + CPH].reshape(4, 128).T.astype(np.float32)
        )
        in_maps.append(
            {
                "xTr": xTr[b],
                "wqr": _pm(W_qkv[:, lo : lo + CPH], 8),
                "wkr": _pm(W_qkv[:, C + lo : C + lo + CPH], 8),
                "wvr": _pm(W_qkv[:, 2 * C + lo : 2 * C + lo + CPH], 8),
                "bqr": bqr,
                "bkr": bkr,
                "wor": _pm(W_out[lo : lo + CPH, :], 4),
                "masks": masks,
            }
        )
    return in_maps


def _gather(results, b_out, bias_extra):
    bias = np.asarray(b_out, dtype=np.float32) + bias_extra
    out = np.empty((B, T, C), np.float32)
    for b in range(B):
        out[b] = (
            results[G * b]["yp"].astype(np.float32)
            + results[G * b + 1]["yp"].astype(np.float32)
            + bias[None, :]
        )
    return out


def kernel(x, W_qkv, b_qkv, W_out, b_out, **_):
    nc = _build_program()
    in_maps = _make_in_maps(x, W_qkv, b_qkv, W_out)
    res = bass_utils.run_bass_kernel_spmd(nc, in_maps, core_ids=list(range(NCORES)))
    bias_extra = np.asarray(b_qkv, np.float32)[2 * C :] @ np.asarray(W_out, np.float32)
    return _gather(res.results, b_out, bias_extra)


def kernel_traced(x, W_qkv, b_qkv, W_out, b_out, tmpdir=None, trace=True, **_):
    """Like kernel() but returns (out, exec_time_ns); used by test.py."""
    nc = _build_program()
    in_maps = _make_in_maps(x, W_qkv, b_qkv, W_out)
    res = bass_utils.run_bass_kernel_spmd(
        nc, in_maps, core_ids=list(range(NCORES)), trace=trace, tmpdir=tmpdir
    )
    bias_extra = np.asarray(b_qkv, np.float32)[2 * C :] @ np.asarray(W_out, np.float32)
    return _gather(res.results, b_out, bias_extra), res.exec_time_ns


# revision 25
# speedup vs baseline: 1.1868x; 1.0016x over previous
"""Causal self-attention (B=4, T=2048, C=1024, H=16, D=64) on 8 trn2 cores.

Sharding: core c -> (batch b = c//2, head-group g = c%2); a head group is
8 heads = 512 feature columns of each of Q/K/V.  Per core, one fully
software-pipelined program:

  - QKV projection blocks produce Q^T/K^T [64,2048] fp16 per head and
    V [2048,64] fp16 (+ a ones column that makes the AV matmul emit the
    softmax denominator for free).
  - Scores stay transposed (S^T[k,q]) so exp(S^T) feeds the AV matmul as
    the moving operand with no transposes anywhere.  On diagonal chunks
    the two heads' valid column ranges are written contiguously so the
    exp is a single wide ACT call, and only the 128-wide causal triangle
    is masked (the fully-masked region is excluded from the matmuls).
  - The attention stream is ACT(exp)-bound, so the next token-block's
    projection matmuls and earlier query-blocks' out-projections are
    drip-fed as 2-matmul micro-chunks inside the attention kc-loop to
    fill TensorE slack.

Host pre-arranges inputs partition-major (fp16) and sums the two per-batch
partials, folding b_out + b_v @ W_out (exact: softmax rows sum to 1).

All matmuls run fp16 with fp32 PSUM accumulation.
"""

from collections import deque
from contextlib import ExitStack

import numpy as np

import concourse.bass as bass
import concourse.mybir as mybir
import concourse.tile as tile
from concourse import bacc
from concourse import bass_utils

F32 = mybir.dt.float32
F16 = mybir.dt.float16

B, T, C = 4, 2048, 1024
H, D = 16, 64
G = 2            # head groups (cores per batch)
HPG = 8          # heads per group
CPH = HPG * D    # feature columns per group = 512
N = 512          # matmul moving free dim
NCORES = 8
SCALE = 1.0 / np.sqrt(D)

_CACHE = {}


def _build_program():
    if "nc" in _CACHE:
        return _CACHE["nc"]

    nc = bacc.Bacc("TRN2", target_bir_lowering=False, debug=False, num_devices=NCORES)

    # all inputs pre-arranged host-side: partition-major, fp16.
    # x is stored token-block-major so each token block is one DMA and the
    # first projection block only depends on the first x DMA.
    xTr = nc.dram_tensor("xTr", [4, 128, 8, N], F16, kind="ExternalInput").ap()
    wqr = nc.dram_tensor("wqr", [128, 8, CPH], F16, kind="ExternalInput").ap()
    wkr = nc.dram_tensor("wkr", [128, 8, CPH], F16, kind="ExternalInput").ap()
    wvr = nc.dram_tensor("wvr", [128, 8, CPH], F16, kind="ExternalInput").ap()
    bqr = nc.dram_tensor("bqr", [128, 4], F32, kind="ExternalInput").ap()
    bkr = nc.dram_tensor("bkr", [128, 4], F32, kind="ExternalInput").ap()
    wor = nc.dram_tensor("wor", [128, 4, C], F16, kind="ExternalInput").ap()
    masks = nc.dram_tensor("masks", [128, 256], F16, kind="ExternalInput").ap()
    yp = nc.dram_tensor("yp", [T, C], F16, kind="ExternalOutput").ap()

    with tile.TileContext(nc) as tc, ExitStack() as ctx:
        wpool = ctx.enter_context(tc.tile_pool(name="wpool", bufs=1))
        big = ctx.enter_context(tc.tile_pool(name="big", bufs=1))
        epool = ctx.enter_context(tc.tile_pool(name="et", bufs=8))
        mpool = ctx.enter_context(tc.tile_pool(name="mpool", bufs=6))
        blkps = ctx.enter_context(tc.tile_pool(name="blkps", bufs=2, space="PSUM"))
        sps = ctx.enter_context(tc.tile_pool(name="sps", bufs=2, space="PSUM"))
        avps = ctx.enter_context(tc.tile_pool(name="avps", bufs=1, space="PSUM"))

        XT = [big.tile([128, 8, N], F16, name=f"xt{tb}") for tb in range(4)]
        QT = big.tile([128, 4, T], F16)   # Q^T (+bias)
        KT = big.tile([128, 4, T], F16)   # SCALE * (K^T + bias)
        VA = big.tile([128, 16, HPG, D + 1], F16)   # V rows + ones column
        ON = big.tile([128, 4, T], F16)   # normalized O^T (c_in x tokens)

        WQ = wpool.tile([128, 8, CPH], F16)
        WK = wpool.tile([128, 8, CPH], F16)
        WV = wpool.tile([128, 8, CPH], F16)
        BQ = wpool.tile([128, 4], F32)
        BKs = wpool.tile([128, 4], F32)
        MS = wpool.tile([128, 256], F16)
        WO = wpool.tile([128, 4, C], F16)

        WARM = wpool.tile([128, N], F16)  # operand for PE warm-up matmuls
        # issued while the first input DMAs stream in, so the HAM
        # clock-gate is at 8/8 when real matmuls start
        nc.any.memset(WARM[:], 1.0)
        warmps = blkps.tile([128, N], F32, name="blk")
        for _ in range(38):
            nc.tensor.matmul(
                warmps[:], WARM[:, 0:128], WARM[:], start=True, stop=True,
                skip_group_check=True,
            )

        # input DMAs in first-use order: the first projection block (q, tb0)
        # needs only WQ + XT[0]
        nc.sync.dma_start(WQ[:], wqr)
        nc.sync.dma_start(XT[0][:], xTr[0])
        nc.sync.dma_start(WK[:], wkr)
        nc.sync.dma_start(WV[:], wvr)
        nc.sync.dma_start(BQ[:], bqr)
        nc.sync.dma_start(BKs[:], bkr)
        nc.vector.tensor_scalar_mul(BKs[:], BKs[:], SCALE)
        # masks split in two so XT[1] is the 9th DMA: it then waits for a
        # free hardware queue slot instead of stealing input bandwidth
        # from WK/WV during the startup-critical window
        nc.sync.dma_start(MS[:, 0:128], masks[:, 0:128])
        nc.sync.dma_start(MS[:, 128:256], masks[:, 128:256])
        nc.any.memset(VA[:, :, :, D : D + 1], 1.0)
        nc.sync.dma_start(XT[1][:], xTr[1])
        nc.sync.dma_start(XT[2][:], xTr[2])
        nc.sync.dma_start(WO[:], wor)
        nc.sync.dma_start(XT[3][:], xTr[3])

        def qkv_block_gen(tb, which, dc):
            """Generator: one projection block, yielding every 2 matmuls."""
            ps = blkps.tile([128, N], F32, name="blk")
            if which == "v":
                for cc in range(8):
                    nc.tensor.matmul(
                        ps[:],
                        XT[tb][:, cc, dc * 128 : (dc + 1) * 128],
                        WV[:, cc],
                        start=(cc == 0),
                        stop=(cc == 7),
                    )
                    if cc % 2 == 1:
                        yield
                nc.vector.tensor_copy(
                    VA[:, tb * 4 + dc, :, 0:D],
                    ps[:].rearrange("p (h d) -> p h d", h=HPG),
                )
            else:
                WT, dst, scl, bias = (
                    (WQ, QT, 1.0, BQ) if which == "q" else (WK, KT, SCALE, BKs)
                )
                for cc in range(8):
                    nc.tensor.matmul(
                        ps[:],
                        WT[:, cc, dc * 128 : (dc + 1) * 128],
                        XT[tb][:, cc, :],
                        start=(cc == 0),
                        stop=(cc == 7),
                    )
                    if cc % 2 == 1:
                        yield
                nc.vector.scalar_tensor_tensor(
                    out=dst[:, dc, tb * N : (tb + 1) * N],
                    in0=ps[:],
                    scalar=scl,
                    in1=bias[:, dc, None].to_broadcast((128, N)),
                    op0=mybir.AluOpType.mult,
                    op1=mybir.AluOpType.add,
                )

        def y_block_gen(ic, ob):
            """Generator: one out-projection block, yielding every 2 matmuls."""
            ypt = blkps.tile([128, N], F32, name="blk")
            for cc4 in range(4):
                nc.tensor.matmul(
                    ypt[:],
                    ON[:, cc4, ic * 128 : (ic + 1) * 128],
                    WO[:, cc4, ob * N : (ob + 1) * N],
                    start=(cc4 == 0),
                    stop=(cc4 == 3),
                )
                if cc4 % 2 == 1:
                    yield
            ysb = mpool.tile([128, N], F16, name="ysb")
            nc.vector.tensor_copy(ysb[:], ypt[:])
            nc.sync.dma_start(
                yp[ic * 128 : (ic + 1) * 128, ob * N : (ob + 1) * N], ysb[:]
            )

        drip = deque()

        def drip_advance(n):
            for _ in range(n):
                while drip:
                    try:
                        next(drip[0])
                        break
                    except StopIteration:
                        drip.popleft()
                else:
                    return

        def drip_drain():
            while drip:
                drip_advance(1)

        def drain_until(g):
            # run the drip until generator g (already in the deque) has
            # completed, so everything it emits lands in program order
            # before the instruction that depends on it
            while g in drip:
                drip_advance(1)

        def ph1_gens(tb):
            # Q chunks first (needed at the start of query-block tb), then
            # K and V (needed from kc=4*tb onwards)
            return (
                [qkv_block_gen(tb, "q", dc) for dc in range(4)]
                + [qkv_block_gen(tb, "k", dc) for dc in range(4)]
                + [qkv_block_gen(tb, "v", dc) for dc in range(4)]
            )

        def y_gens(qi):
            return [
                y_block_gen(4 * qi + i4, ob) for i4 in range(4) for ob in range(2)
            ]

        # token-block 0 preamble: only the blocks attention (qi=0, pr=0)
        # needs; the remaining head-pairs' q/k blocks drip inside qi=0.
        pre0 = [("q", 0), ("k", 0), ("v", 0), ("v", 1), ("v", 2), ("v", 3)]
        drip.extend(qkv_block_gen(0, w, dc) for w, dc in pre0)
        drip_drain()
        qk0 = {(w, dc): qkv_block_gen(0, w, dc)
               for w, dc in [("q", 1), ("k", 1), ("q", 2), ("k", 2),
                             ("q", 3), ("k", 3)]}
        drip.extend(qk0.values())
        q3 = {}
        kv3 = {}

        # drip rate per attention iteration, as a fraction (num/den).
        # qi=0/1 are PE-bound (mandatory next-token-block projections), so
        # the out-projection backlog is deferred to qi=2/3 where the
        # ACT-paced attention stream leaves TensorE slack.
        # Supply (generator next()-calls: a qkv block is 5, a y block is 3):
        #   qi=0 (16 it): rest of tb0 (30) + tb1 (60)
        #   qi=1 (32 it): tb2 (60)
        #   qi=2 (48 it): tb3 q-blocks (20) + y(qi0) (24) + y(qi1) (24)
        #   qi=3 (64 it): tb3 k/v (40, first used at kc=12) + y(qi2) (24)
        DRIP_BUDGET = {0: (6, 1), 1: (2, 1), 2: (3, 2), 3: (1, 1)}
        for qi in range(4):
            nkc = 4 * qi + 4
            if qi in (0, 1):
                drip_drain()  # safety: previous token block must be complete
                drip.extend(ph1_gens(qi + 1))
            elif qi == 2:
                drip_drain()
                # only tb3's Q is needed at the start of qi=3; its K/V
                # blocks drip inside qi=3 (first used at kc=12)
                q3 = {dc: qkv_block_gen(3, "q", dc) for dc in range(4)}
                drip.extend(q3.values())
                drip.extend(y_gens(0))
                drip.extend(y_gens(1))
            else:
                kv3 = {(w, dc): qkv_block_gen(3, w, dc)
                       for w, dc in [("k", 0), ("v", 0), ("v", 1), ("v", 2),
                                     ("v", 3), ("k", 1), ("k", 2), ("k", 3)]}
                drip.extend(kv3.values())
                drip.extend(y_gens(2))
            bnum, bden = DRIP_BUDGET[qi]
            bacc_ctr = 0
            for pr in range(4):
                # force-complete the projection blocks this head-pair's
                # attention reads, so their instructions precede ours
                if qi == 0 and pr > 0:
                    drain_until(qk0[("q", pr)])
                    drain_until(qk0[("k", pr)])
                if qi == 3:
                    drain_until(q3[pr])
                avs = []
                for hi in range(2):
                    av = avps.tile([D + 1, N], F32, name=f"av{hi}")
                    avs.append(av)

                def emit_s(kc, pr=pr, qi=qi):
                    # both heads' score tiles in one 2-bank psum tile so the
                    # exp runs wide; the two matmuls run concurrently (row
                    # groups 0-1 / 2-3).  Diagonal chunks only compute the
                    # causally-reachable range w = N - vq; head 0 is packed
                    # at the END of psum bank 0 ([N-w, N)) and head 1 at the
                    # START of bank 1 ([N, N+w)) so the valid region is one
                    # contiguous [N-w, N+w) stripe while each matmul output
                    # stays inside a single psum bank.
                    vq = max(0, (kc - 4 * qi) * 128)
                    w = N - vq
                    sp = sps.tile([128, 2 * N], F32, name="sp")
                    for hi in range(2):
                        off = 64 * hi
                        nc.tensor.matmul(
                            sp[:, N - w + hi * w : N + hi * w],
                            KT[off : off + 64, pr, kc * 128 : (kc + 1) * 128],
                            QT[off : off + 64, pr, qi * N + vq : (qi + 1) * N],
                            start=True,
                            stop=True,
                        )
                    return sp

                sp_cur = emit_s(0)
                for kc in range(nkc):
                    et = epool.tile([128, 2 * N], F16, name="et")
                    vq = max(0, (kc - 4 * qi) * 128)
                    w = N - vq
                    nc.scalar.activation(
                        et[:, N - w : N + w],
                        sp_cur[:, N - w : N + w],
                        mybir.ActivationFunctionType.Exp,
                    )
                    if kc >= 4 * qi:
                        # only the leading 128 columns of each head's range
                        # are partially masked (local query j vs key k:
                        # masked iff j < k); the rest is fully visible.
                        if w == 128:
                            nc.vector.tensor_tensor(
                                et[:, N - 128 : N + 128],
                                et[:, N - 128 : N + 128],
                                MS[:],
                                mybir.AluOpType.mult,
                            )
                        else:
                            for hi in range(2):
                                lo = N - w + hi * w
                                nc.vector.tensor_tensor(
                                    et[:, lo : lo + 128],
                                    et[:, lo : lo + 128],
                                    MS[:, 0:128],
                                    mybir.AluOpType.mult,
                                )
                    for hi in range(2):
                        nc.tensor.matmul(
                            avs[hi][:, vq:],
                            VA[:, kc, 2 * pr + hi, :],
                            et[:, N - w + hi * w : N + hi * w],
                            start=(kc == 0),
                            stop=(kc == nkc - 1),
                        )
                    if qi == 3 and 12 <= kc + 1 < nkc:
                        # tb3 K/V blocks drip inside qi=3; force-complete
                        # the ones the next chunk's S and AV read
                        if kc + 1 == 12:
                            drain_until(kv3[("k", pr)])
                        drain_until(kv3[("v", kc + 1 - 12)])
                    sp_cur = emit_s(kc + 1) if kc + 1 < nkc else None
                    bacc_ctr += bnum
                    drip_advance(bacc_ctr // bden)
                    bacc_ctr %= bden

                last_pr = qi == 3 and pr == 3
                for hi in range(2):
                    off = 64 * hi
                    if last_pr:
                        # nothing reuses the accumulator banks after this,
                        # so skip the bank-releasing copy and read PSUM
                        # directly — shortens the final-normalize tail
                        oc = avs[hi]
                    else:
                        # one copy releases the accumulator bank; the rest
                        # of the chain runs off SBUF, off the critical path
                        oc = mpool.tile([D + 1, N], F32, name="oc")
                        nc.vector.tensor_copy(oc[:], avs[hi][:])
                    dn = mpool.tile([1, N], F32, name="dn")
                    nc.vector.tensor_copy(dn[:], oc[D : D + 1, :])
                    rd = mpool.tile([1, N], F32, name="rd")
                    nc.vector.reciprocal_approx_fast(rd[:], dn[:])
                    rb = mpool.tile([64, N], F32, name="rb")
                    nc.gpsimd.partition_broadcast(rb[:], rd[:])
                    seg = ON[off : off + 64, pr, qi * N : (qi + 1) * N]
                    nc.vector.tensor_tensor(
                        seg, oc[0:64, :], rb[:], mybir.AluOpType.mult
                    )
        drip.extend(y_gens(3))
        drip_drain()

    nc.compile()
    _CACHE["nc"] = nc
    return nc


def _make_masks():
    kp = np.arange(128)[:, None]
    qf = np.arange(128)[None, :]
    tri = (qf >= kp)
    m = np.concatenate([tri, tri], axis=1)  # [128, 256]
    return np.ascontiguousarray(m.astype(np.float16))


def _pm(a, chunks):
    """[chunks*128, F] -> partition-major [128, chunks, F] fp16, contiguous."""
    f = a.shape[-1]
    return np.ascontiguousarray(
        a.reshape(chunks, 128, f).transpose(1, 0, 2).astype(np.float16)
    )


def _make_in_maps(x, W_qkv, b_qkv, W_out):
    x = np.asarray(x, dtype=np.float32)
    W_qkv = np.asarray(W_qkv, dtype=np.float32)
    b_qkv = np.asarray(b_qkv, dtype=np.float32)
    W_out = np.asarray(W_out, dtype=np.float32)
    masks = _make_masks()
    # x[b].T is [C, T]; layout [tb, p, cc, t] with C = cc*128 + p,
    # T = tb*512 + t
    xTr = [
        np.ascontiguousarray(
            x[b].T.reshape(8, 128, 4, N).transpose(2, 1, 0, 3).astype(np.float16)
        )
        for b in range(B)
    ]
    in_maps = []
    for c in range(NCORES):
        b, g = c // G, c % G
        lo = CPH * g
        bqr = np.ascontiguousarray(
            b_qkv[lo : lo + CPH].reshape(4, 128).T.astype(np.float32)
        )
        bkr = np.ascontiguousarray(
            b_qkv[C + lo : C + lo + CPH].reshape(4, 128).T.astype(np.float32)
        )
        in_maps.append(
            {
                "xTr": xTr[b],
                "wqr": _pm(W_qkv[:, lo : lo + CPH], 8),
                "wkr": _pm(W_qkv[:, C + lo : C + lo + CPH], 8),
                "wvr": _pm(W_qkv[:, 2 * C + lo : 2 * C + lo + CPH], 8),
                "bqr": bqr,
                "bkr": bkr,
                "wor": _pm(W_out[lo : lo + CPH, :], 4),
                "masks": masks,
            }
        )
    return in_maps


def _gather(results, b_out, bias_extra):
    bias = np.asarray(b_out, dtype=np.float32) + bias_extra
    out = np.empty((B, T, C), np.float32)
    for b in range(B):
        out[b] = (
            results[G * b]["yp"].astype(np.float32)
            + results[G * b + 1]["yp"].astype(np.float32)
            + bias[None, :]
        )
    return out


def kernel(x, W_qkv, b_qkv, W_out, b_out, **_):
    nc = _build_program()
    in_maps = _make_in_maps(x, W_qkv, b_qkv, W_out)
    res = bass_utils.run_bass_kernel_spmd(nc, in_maps, core_ids=list(range(NCORES)))
    bias_extra = np.asarray(b_qkv, np.float32)[2 * C :] @ np.asarray(W_out, np.float32)
    return _gather(res.results, b_out, bias_extra)


def kernel_traced(x, W_qkv, b_qkv, W_out, b_out, tmpdir=None, trace=True, **_):
    """Like kernel() but returns (out, exec_time_ns); used by test.py."""
    nc = _build_program()
    in_maps = _make_in_maps(x, W_qkv, b_qkv, W_out)
    res = bass_utils.run_bass_kernel_spmd(
        nc, in_maps, core_ids=list(range(NCORES)), trace=trace, tmpdir=tmpdir
    )
    bias_extra = np.asarray(b_qkv, np.float32)[2 * C :] @ np.asarray(W_out, np.float32)
    return _gather(res.results, b_out, bias_extra), res.exec_time_ns
